# revision 22
# baseline (speedup 1.0000x reference)
"""Forward-Forward inference kernel for TRN2 (8 NeuronCores, data-parallel).

Reference math per label l in 0..9:
  h0 = x with cols 0..9 zeroed, col l = max(x); per layer: h <- relu(W @ (h/(||h||+eps)) + b)
  goodness[l] = sum over the 3 layers of mean(h^2); out = argmax_l goodness -> int32 [B]

Device scheme (per core: 1024 rows, two 512-row blocks, labels sequential):
  - Activations transposed: hT [features(partitions) x rows(free)]; weights are
    the stationary matmul operand so no transposes are ever needed.
  - Precision (chosen by CPU bit-exact simulation of the full pipeline):
    L2 moving operand split hi/lo fp16 (2 matmuls, fp32 PSUM accumulate,
    ~22 activation mantissa bits), L3 single fp16, ssq/goodness from the
    PRE-cast fp32 relu output as an exact f16 hi/lo pair of h^2.  Weights are
    single-rounded fp16 (rounding shared across labels => benign for argmax).
  - Layer 1 never touches the PE: the label overlay is rank-1 and the input
    norm is label-independent, so z1 = ybase + xmax*W1[:,l] with ybase built
    once per block and kept in SBUF; per label DVE does (yb + c_l) * s0 and
    ACT applies bias+relu.  Label l+1's layer-1 steps are interleaved into
    label l's L2/L3 of-loops so the PE never waits on them.
  - ssq = ones-matmul over h^2; scale s = 1/(sqrt(ssq)+eps) is applied after
    the next matmul (z = s*(W@h) + b): DVE multiply on the PSUM drain, ACT
    fuses bias+relu.  s is broadcast across partitions by GPSIMD
    partition_broadcast (NOT a PE matmul: that would stall the in-order PE
    stream on the sqrt->reciprocal chain at every layer boundary).
  - argmax: goodness [16, rows] -> PE transpose -> reduce_max / is_equal /
    descending-weight max -> first-max index as int32.
"""

import numpy as np

import concourse.bass as bass
import concourse.mybir as mybir
import concourse.tile as tile
from concourse.bass_utils import run_bass_kernel_spmd
from concourse.masks import make_identity

F16 = mybir.dt.float16
F32 = mybir.dt.float32
I32 = mybir.dt.int32
AF = mybir.ActivationFunctionType
OP = mybir.AluOpType

B, D_IN, H, NL = 8192, 784, 2048, 10
DP = 896                  # D_IN padded to 7*128
EPS = 1e-4
NCORES = 8
BC = B // NCORES          # rows per core
BLK = 512                 # rows per block
NKX = DP // 128           # input chunks for layer 1
NOF = H // 128            # output-feature chunks per layer
NK = H // 128             # input chunks for the 2048-wide layers

# Precision config (validated against the fp32 reference by CPU simulation
# and on hardware: single-f16 moving operands measure rel_err 1.45e-2 =
# 6/8192 label flips, well under the 2e-2 gate; hi/lo options kept for
# fallback)
L2_LO = False
L3_LO = False
SSQ_EXACT = False


def split_sync_waits(nc, max_waits=1):
    """Walrus here accepts at most `max_waits` sync waits per instruction.

    Tile emits instructions waiting on several semaphores at once.  For each
    such instruction, carry the excess waits on same-engine NoOps inserted
    immediately before it: the engine's sequencer executes them in order, so
    all waits still complete before the instruction runs (DMA instructions
    keep one wait in their descriptor; the NoOp just delays the ring).
    """
    uid = [0]
    for f in nc.m.functions:
        for bb in f.blocks:
            out = []
            changed = False
            for ins in bb.instructions:
                si = ins.sync_info
                if si is not None and len(si.on_wait) > max_waits:
                    waits = list(si.on_wait)
                    extra = waits[: len(waits) - max_waits]
                    for i in range(0, len(extra), max_waits):
                        uid[0] += 1
                        nop = mybir.InstNoOp(
                            name=f"waitsplit-{uid[0]}", engine=ins.engine,
                            ins=[], outs=[],
                        )
                        nop.sync_info = mybir.SyncInfo(
                            on_wait=extra[i : i + max_waits], on_update=[]
                        )
                        out.append(nop)
                    ins.sync_info = mybir.SyncInfo(
                        on_wait=waits[len(waits) - max_waits :],
                        on_update=list(si.on_update),
                    )
                    changed = True
                out.append(ins)
            if changed:
                bb.instructions = out


def build_nc(bc=BC, blk=BLK, h=H, l2_lo=L2_LO, l3_lo=L3_LO, ssq_exact=SSQ_EXACT):
    nblk = bc // blk
    ncol = blk // 128
    NOF = h // 128
    NK = h // 128
    nc = bass.Bass()

    xhi_d = nc.dram_tensor("xhi", [DP, bc], F16, kind="ExternalInput")
    xlo_d = nc.dram_tensor("xlo", [DP, bc], F16, kind="ExternalInput")
    s0_d = nc.dram_tensor("s0", [1, bc], F32, kind="ExternalInput")
    # w1g: [of, p, k, j] = W1.T[k*128+p, of*128+j] (per-of contiguous, padded)
    w1g_d = nc.dram_tensor("w1g", [NOF, 128, NKX, 128], F16, kind="ExternalInput")
    w2g_d = nc.dram_tensor("w2g", [NOF, 128, NK, 128], F16, kind="ExternalInput")
    w3g_d = nc.dram_tensor("w3g", [NOF, 128, NK, 128], F16, kind="ExternalInput")
    # w1ct[p, of*NL + l] = xmax * W1[of*128+p, l]  (fp32, exact)
    w1ct_d = nc.dram_tensor("w1ct", [128, NOF * NL], F32, kind="ExternalInput")
    b1_d = nc.dram_tensor("b1r", [NOF, 128], F32, kind="ExternalInput")
    b2_d = nc.dram_tensor("b2r", [NOF, 128], F32, kind="ExternalInput")
    b3_d = nc.dram_tensor("b3r", [NOF, 128], F32, kind="ExternalInput")
    out_d = nc.dram_tensor("out", [bc], I32, kind="ExternalOutput")

    from contextlib import ExitStack

    with tile.TileContext(nc) as tc:
        with ExitStack() as ctx:
            ec = ctx.enter_context
            wstr = ec(tc.tile_pool(name="wstr", bufs=4))
            w1o = ec(tc.tile_pool(name="w1o", bufs=2))
            xs = ec(tc.tile_pool(name="xs", bufs=1))
            const = ec(tc.tile_pool(name="const", bufs=1))
            ybs = ec(tc.tile_pool(name="ybs", bufs=1))
            hhp = ec(tc.tile_pool(name="hhp", bufs=6))
            hlp = ec(tc.tile_pool(name="hlp", bufs=2))
            f32t = ec(tc.tile_pool(name="f32t", bufs=2))
            hf = ec(tc.tile_pool(name="hf", bufs=2))
            h2s = ec(tc.tile_pool(name="h2s", bufs=3))
            sbc = ec(tc.tile_pool(name="sbc", bufs=4))
            s0p = ec(tc.tile_pool(name="s0p", bufs=2))
            srow = ec(tc.tile_pool(name="srow", bufs=2))
            srp = ec(tc.tile_pool(name="srp", bufs=3))
            glp = ec(tc.tile_pool(name="glp", bufs=4))
            gp = ec(tc.tile_pool(name="gp", bufs=2))
            outp = ec(tc.tile_pool(name="outp", bufs=2))
            zp = ec(tc.tile_pool(name="zp", bufs=4, space="PSUM"))
            ssqp = ec(tc.tile_pool(name="ssqp", bufs=4, space="PSUM"))
            # ---- constants --------------------------------------------------
            w1ct = const.tile([128, NOF, NL], F32, tag="w1ct")
            nc.sync.dma_start(
                w1ct[:, :, :], w1ct_d[:, :].rearrange("p (o l) -> p o l", l=NL)
            )
            bt = {}
            for li, bd in ((1, b1_d), (2, b2_d), (3, b3_d)):
                t = const.tile([128, NOF], F32, tag=f"b{li}")
                nc.gpsimd.dma_start(t[:, :], bd[:, :].rearrange("c p -> p c"))
                bt[li] = t
            ident16 = const.tile([16, 16], F32, tag="id16")
            make_identity(nc, ident16[:, :])
            ones_col = const.tile([128, 1], F16, tag="onec")
            nc.vector.memset(ones_col[:, :], 1.0)
            ones_row = const.tile([1, 128], F32, tag="oner")
            nc.vector.memset(ones_row[:, :], 1.0)
            desc_i = const.tile([128, 16], I32, tag="desci")
            nc.gpsimd.iota(desc_i[:, :], [[-1, 16]], base=16, channel_multiplier=0)
            desc_f = const.tile([128, 16], F32, tag="descf")
            nc.vector.tensor_copy(desc_f[:, :], desc_i[:, :])
            s0row = const.tile([1, bc], F32, tag="s0row")
            nc.sync.dma_start(s0row[:, :], s0_d[:, :])

            for blki in range(nblk):
                r0 = blki * blk

                # per-label state: sb (bcast scale), g_lab, h tiles
                st = {}

                def chain_to_sr(ssq):
                    """sr = 1/(sqrt(ssq)+eps) as a [1, blk] row (no broadcast)."""
                    u = srow.tile([1, blk], F32, tag="u")
                    nc.scalar.activation(u[:, :], ssq[:, :], AF.Sqrt)
                    ue = srow.tile([1, blk], F32, tag="ue")
                    nc.vector.tensor_scalar_add(ue[:, :], u[:, :], EPS)
                    sr = srp.tile([1, blk], F32, tag="sr")
                    nc.vector.reciprocal(sr[:, :], ue[:, :])
                    return sr

                def bcast_row(row_ap, pool=None):
                    """[1, blk] -> [128, blk] via ones-matmul; DVE drains PSUM.
                    Emit AFTER some independent PE work so the in-order PE does
                    not idle on the reciprocal chain."""
                    ps = zp.tile([128, blk], F32, tag="z", name="sbps")
                    nc.tensor.matmul(ps[:, :], ones_row[:, :], row_ap,
                                     start=True, stop=True)
                    sb = (pool or sbc).tile([128, blk], F32, tag="sb", name="sb")
                    nc.vector.tensor_copy(sb[:, :], ps[:, :])
                    return sb

                s0b = bcast_row(s0row[0:1, r0 : r0 + blk], pool=s0p)

                g_all = gp.tile([16, blk], F32, tag="gall")
                # rows 0..9 are overwritten per label; 10..15 stay -1e30
                nc.vector.memset(g_all[:, :], -1e30)

                def drain_and_square(lab, layer, of, tin, n_hi, n_lo, ssq):
                    """relu/cast/square pipeline shared by all layers.
                    tin: fp32 [128, blk] pre-bias pre-relu input (already scaled).
                    """
                    hv = hf.tile([128, blk], F32, tag="hv")
                    nc.scalar.activation(
                        hv[:, :], tin[:, :], AF.Relu,
                        bias=bt[layer][:, of : of + 1], scale=1.0,
                    )
                    if n_hi is not None:
                        nc.vector.tensor_copy(n_hi[:, of, :], hv[:, :])
                        if n_lo is not None:
                            nc.vector.tensor_tensor(
                                out=n_lo[:, of, :], in0=hv[:, :],
                                in1=n_hi[:, of, :], op=OP.subtract,
                            )
                    # ssq from the PRE-cast fp32 hv (precision-critical)
                    if ssq_exact:
                        h2 = f32t.tile([128, blk], F32, tag="h2")
                        nc.scalar.activation(h2[:, :], hv[:, :], AF.Square)
                        h2hi = h2s.tile([128, blk], F16, tag="h2hi")
                        nc.scalar.activation(h2hi[:, :], h2[:, :], AF.Copy)
                        h2lo = h2s.tile([128, blk], F16, tag="h2lo")
                        nc.vector.tensor_tensor(
                            out=h2lo[:, :], in0=h2[:, :], in1=h2hi[:, :],
                            op=OP.subtract,
                        )
                        nc.tensor.matmul(
                            ssq[:, :], ones_col[:, :], h2hi[:, :],
                            start=(of == 0), stop=False,
                        )
                        nc.tensor.matmul(
                            ssq[:, :], ones_col[:, :], h2lo[:, :],
                            start=False, stop=(of == NOF - 1),
                        )
                    else:
                        h2hi = h2s.tile([128, blk], F16, tag="h2hi")
                        nc.scalar.activation(h2hi[:, :], hv[:, :], AF.Square)
                        nc.tensor.matmul(
                            ssq[:, :], ones_col[:, :], h2hi[:, :],
                            start=(of == 0), stop=(of == NOF - 1),
                        )

                def l1_start(lab):
                    s = st[lab] = {}
                    s["n_hi"] = hhp.tile([128, NK, blk], F16, tag="hhi", name="n_hi")
                    s["n_lo"] = (
                        hlp.tile([128, NK, blk], F16, tag="hlo", name="n_lo")
                        if l2_lo
                        else None
                    )
                    s["ssq"] = ssqp.tile([1, blk], F32, tag="ssq", name="ssq")

                def l1_step(lab, of):
                    s = st[lab]
                    t0 = f32t.tile([128, blk], F32, tag="t0")
                    nc.vector.tensor_scalar(
                        out=t0[:, :], in0=ybt[:, of, :],
                        scalar1=w1ct[:, of, lab : lab + 1],
                        scalar2=None, op0=OP.add,
                    )
                    t = f32t.tile([128, blk], F32, tag="t")
                    nc.vector.tensor_tensor(
                        out=t[:, :], in0=t0[:, :], in1=s0b[:, :], op=OP.mult
                    )
                    drain_and_square(lab, 1, of, t, s["n_hi"], s["n_lo"], s["ssq"])

                def l1_finish(lab):
                    s = st[lab]
                    g_lab = glp.tile([1, blk], F32, tag="glab")
                    nc.vector.tensor_copy(g_lab[:, :], s["ssq"][:, :])
                    s["g"] = g_lab
                    s["sr"] = chain_to_sr(s["ssq"])

                # ---- ybase = W1 @ x_zeroed for this block, SBUF-resident ----
                # (layer 1 of label 0 rides along, one of-chunk at a time)
                ybt = ybs.tile([128, NOF, blk], F16, tag="ybt")
                xh, xl = [], []
                for ki in range(NKX):
                    th = xs.tile([128, blk], F16, tag=f"xh{ki}")
                    tl = xs.tile([128, blk], F16, tag=f"xl{ki}")
                    ko = ki * 128
                    nc.sync.dma_start(th[:, :], xhi_d[ko : ko + 128, r0 : r0 + blk])
                    nc.sync.dma_start(tl[:, :], xlo_d[ko : ko + 128, r0 : r0 + blk])
                    xh.append(th)
                    xl.append(tl)
                l1_start(0)
                l1_start(1)
                for of in range(NOF):
                    w1of = w1o.tile([128, NKX, 128], F16, tag="w1of")
                    nc.sync.dma_start(w1of[:, :, :], w1g_d[of, :, :, :])
                    ps = zp.tile([128, blk], F32, tag="z")
                    for ki in range(NKX):
                        nc.tensor.matmul(
                            ps[:, :], w1of[:, ki, :], xh[ki][:, :],
                            start=(ki == 0), stop=False,
                        )
                        nc.tensor.matmul(
                            ps[:, :], w1of[:, ki, :], xl[ki][:, :],
                            start=False, stop=(ki == NKX - 1),
                        )
                    nc.scalar.activation(ybt[:, of, :], ps[:, :], AF.Copy)
                    l1_step(0, of)
                    l1_step(1, of)
                l1_finish(0)
                l1_finish(1)

                # ---- labels in pairs: both share each stationary weight load,
                # so the PE streams back-to-back and LDWEIGHTS fully hides ----
                for pr in range(NL // 2):
                    A, B = 2 * pr, 2 * pr + 1
                    carry = pr + 1 < NL // 2
                    if carry:
                        l1_start(A + 2)
                        l1_start(B + 2)
                    hh = {A: st[A]["n_hi"], B: st[B]["n_hi"]}
                    hl = {A: st[A]["n_lo"], B: st[B]["n_lo"]}
                    for layer in (2, 3):
                        wg_d = w2g_d if layer == 2 else w3g_d
                        in_lo = (layer == 2 and l2_lo) or (layer == 3 and l3_lo)
                        out_lo = layer == 2 and l3_lo
                        nh = {A: None, B: None}
                        nl_ = {A: None, B: None}
                        if layer == 2:
                            for lab in (A, B):
                                t_ = hhp.tile(
                                    [128, NK, blk], F16, tag="hhi", name="n_hi"
                                )
                                nh[lab] = t_
                                if out_lo:
                                    t2_ = hlp.tile(
                                        [128, NK, blk], F16, tag="hlo", name="n_lo"
                                    )
                                    nl_[lab] = t2_
                        ssq = {}
                        for lab in (A, B):
                            ssq[lab] = ssqp.tile([1, blk], F32, tag="ssq", name="ssq")
                        sb = {A: None, B: None}
                        pend = []

                        def drain_one():
                            dof, dps = pend.pop(0)
                            for lab in (A, B):
                                t = f32t.tile([128, blk], F32, tag="t")
                                nc.vector.tensor_tensor(
                                    out=t[:, :], in0=dps[lab][:, :],
                                    in1=sb[lab][:, :], op=OP.mult,
                                )
                                drain_and_square(
                                    lab, layer, dof, t, nh[lab], nl_[lab], ssq[lab]
                                )

                        for of in range(NOF):
                            wof = wstr.tile([128, NK, 128], F16, tag="wof")
                            nc.sync.dma_start(wof[:, :, :], wg_d[of, :, :, :])
                            ps = {}
                            for lab in (A, B):
                                ps[lab] = zp.tile([128, blk], F32, tag="z", name="z")
                            for k in range(NK):
                                for lab in (A, B):
                                    nc.tensor.matmul(
                                        ps[lab][:, :], wof[:, k, :], hh[lab][:, k, :],
                                        start=(k == 0),
                                        stop=(k == NK - 1 and not in_lo),
                                    )
                                    if in_lo:
                                        nc.tensor.matmul(
                                            ps[lab][:, :], wof[:, k, :],
                                            hl[lab][:, k, :],
                                            start=False, stop=(k == NK - 1),
                                        )
                            pend.append((of, ps))
                            if of == 0:
                                # broadcast AFTER of0's paired mains (~7us): the
                                # in-order PE covers the reciprocal-chain
                                # latency with real work (drains lag 1 behind)
                                sb[A] = bcast_row(st[A]["sr"][0:1, :])
                                sb[B] = bcast_row(st[B]["sr"][0:1, :])
                            if of >= 1:
                                drain_one()
                            # layer-1 of the NEXT pair rides along in layer 2
                            # (so its ssq PSUM group closes before L3 starts)
                            if carry and layer == 2:
                                l1_step(A + 2, of)
                                l1_step(B + 2, of)
                        while pend:
                            drain_one()
                        for lab in (A, B):
                            nc.vector.tensor_tensor(
                                out=st[lab]["g"][:, :], in0=st[lab]["g"][:, :],
                                in1=ssq[lab][:, :], op=OP.add,
                            )
                        if layer == 2:
                            for lab in (A, B):
                                st[lab]["sr"] = chain_to_sr(ssq[lab])
                            if carry:
                                l1_finish(A + 2)
                                l1_finish(B + 2)
                            hh, hl = nh, nl_
                    for lab in (A, B):
                        nc.gpsimd.dma_start(
                            g_all[lab : lab + 1, :], st[lab]["g"][:, :]
                        )
                        del st[lab]

                # ---- argmax over labels -------------------------------------
                for c in range(ncol):
                    cs = slice(c * 128, (c + 1) * 128)
                    gt = zp.tile([128, blk], F32, tag="z", name="gt")
                    nc.tensor.transpose(gt[:, 0:16], g_all[:, cs], ident16[:, :])
                    mx = outp.tile([128, 1], F32, tag="mx")
                    nc.vector.reduce_max(
                        out=mx[:, :], in_=gt[:, 0:16], axis=mybir.AxisListType.X
                    )
                    eq = outp.tile([128, 16], F32, tag="eq")
                    nc.vector.tensor_scalar(
                        out=eq[:, :], in0=gt[:, 0:16], scalar1=mx[:, :],
                        scalar2=None, op0=OP.is_equal,
                    )
                    wv = outp.tile([128, 16], F32, tag="wv")
                    nc.vector.tensor_tensor(
                        out=wv[:, :], in0=eq[:, :], in1=desc_f[:, :], op=OP.mult
                    )
                    vv = outp.tile([128, 1], F32, tag="vv")
                    nc.vector.reduce_max(
                        out=vv[:, :], in_=wv[:, :], axis=mybir.AxisListType.X
                    )
                    idx = outp.tile([128, 1], I32, tag="idx")
                    nc.vector.tensor_scalar(
                        out=idx[:, :], in0=vv[:, :], scalar1=-1.0, scalar2=16.0,
                        op0=OP.mult, op1=OP.add,
                    )
                    nc.sync.dma_start(
                        out_d[r0 + c * 128 : r0 + (c + 1) * 128], idx[:, :]
                    )
    split_sync_waits(nc)
    return nc


def prep_inputs(x, W1, b1, W2, b2, W3, b3, ncores=NCORES, h=H):
    """Host-side marshaling: overlay zeroing, norms of the fixed input rows,
    transposes, fp16 hi/lo split of x, weight regrouping, per-core sharding."""
    x = np.asarray(x, dtype=np.float32)
    bc = x.shape[0] // ncores
    xmax = np.float32(x.max())
    x_ = x.copy()
    x_[:, :NL] = 0.0
    ssq0 = (x_ * x_).sum(axis=1, dtype=np.float32) + xmax * xmax
    s0 = (np.float32(1.0) / (np.sqrt(ssq0) + np.float32(EPS))).astype(np.float32)

    xp = np.zeros((x.shape[0], DP), np.float32)
    xp[:, :D_IN] = x_
    xT = np.ascontiguousarray(xp.T)                      # [896, B] fp32
    xhi = xT.astype(np.float16)
    xlo = (xT - xhi.astype(np.float32)).astype(np.float16)

    NOF = NK = h // 128

    def regroup(Wt, nk):
        # Wt: [in, out] fp16 -> [of, p, k, j]
        return np.ascontiguousarray(
            Wt.reshape(nk, 128, NOF, 128).transpose(2, 1, 0, 3)
        )

    w1p = np.zeros((DP, h), np.float16)
    w1p[:D_IN] = W1.T.astype(np.float16)
    w1g = regroup(w1p, NKX)
    w2g = regroup(W2.T.astype(np.float16), NK)
    w3g = regroup(W3.T.astype(np.float16), NK)
    # w1ct[p, of, l] = xmax * W1[of*128+p, l], fp32 exact
    w1ct = np.ascontiguousarray(
        (xmax * W1[:, :NL].astype(np.float32)).reshape(NOF, 128, NL).transpose(1, 0, 2)
    ).reshape(128, NOF * NL)
    b1r = np.ascontiguousarray(b1.reshape(NOF, 128)).astype(np.float32)
    b2r = np.ascontiguousarray(b2.reshape(NOF, 128)).astype(np.float32)
    b3r = np.ascontiguousarray(b3.reshape(NOF, 128)).astype(np.float32)

    in_maps = []
    for c in range(ncores):
        rs = slice(c * bc, (c + 1) * bc)
        in_maps.append(
            {
                "xhi": np.ascontiguousarray(xhi[:, rs]),
                "xlo": np.ascontiguousarray(xlo[:, rs]),
                "s0": np.ascontiguousarray(s0[rs]).reshape(1, bc),
                "w1g": w1g, "w2g": w2g, "w3g": w3g, "w1ct": w1ct,
                "b1r": b1r, "b2r": b2r, "b3r": b3r,
            }
        )
    return in_maps, bc


_NC_CACHE = {}


def kernel(x, W1, b1, W2, b2, W3, b3, trace=False):
    in_maps, bc = prep_inputs(x, W1, b1, W2, b2, W3, b3)
    if "nc" not in _NC_CACHE:
        _NC_CACHE["nc"] = build_nc(bc=bc)
    res = run_bass_kernel_spmd(
        _NC_CACHE["nc"], in_maps, core_ids=list(range(NCORES)), trace=trace
    )
    out = np.concatenate([res.results[c]["out"] for c in range(NCORES)])
    if trace:
        kernel.last_results = res
    return out


# revision 25
# speedup vs baseline: 1.0090x; 1.0090x over previous
"""Forward-Forward inference kernel for TRN2 (8 NeuronCores, data-parallel).

Reference math per label l in 0..9:
  h0 = x with cols 0..9 zeroed, col l = max(x); per layer: h <- relu(W @ (h/(||h||+eps)) + b)
  goodness[l] = sum over the 3 layers of mean(h^2); out = argmax_l goodness -> int32 [B]

Device scheme (per core: 1024 rows, two 512-row blocks, labels sequential):
  - Activations transposed: hT [features(partitions) x rows(free)]; weights are
    the stationary matmul operand so no transposes are ever needed.
  - Precision (chosen by CPU bit-exact simulation of the full pipeline):
    L2 moving operand split hi/lo fp16 (2 matmuls, fp32 PSUM accumulate,
    ~22 activation mantissa bits), L3 single fp16, ssq/goodness from the
    PRE-cast fp32 relu output as an exact f16 hi/lo pair of h^2.  Weights are
    single-rounded fp16 (rounding shared across labels => benign for argmax).
  - Layer 1 never touches the PE: the label overlay is rank-1 and the input
    norm is label-independent, so z1 = ybase + xmax*W1[:,l] with ybase built
    once per block and kept in SBUF; per label DVE does (yb + c_l) * s0 and
    ACT applies bias+relu.  Label l+1's layer-1 steps are interleaved into
    label l's L2/L3 of-loops so the PE never waits on them.
  - ssq = ones-matmul over h^2; scale s = 1/(sqrt(ssq)+eps) is applied after
    the next matmul (z = s*(W@h) + b): DVE multiply on the PSUM drain, ACT
    fuses bias+relu.  s is broadcast across partitions by GPSIMD
    partition_broadcast (NOT a PE matmul: that would stall the in-order PE
    stream on the sqrt->reciprocal chain at every layer boundary).
  - argmax: goodness [16, rows] -> PE transpose -> reduce_max / is_equal /
    descending-weight max -> first-max index as int32.
"""

import numpy as np

import concourse.bass as bass
import concourse.mybir as mybir
import concourse.tile as tile
from concourse.bass_utils import run_bass_kernel_spmd
from concourse.masks import make_identity

F16 = mybir.dt.float16
F32 = mybir.dt.float32
I32 = mybir.dt.int32
AF = mybir.ActivationFunctionType
OP = mybir.AluOpType

B, D_IN, H, NL = 8192, 784, 2048, 10
DP = 896                  # D_IN padded to 7*128
EPS = 1e-4
NCORES = 8
BC = B // NCORES          # rows per core
BLK = 512                 # rows per block
NKX = DP // 128           # input chunks for layer 1
NOF = H // 128            # output-feature chunks per layer
NK = H // 128             # input chunks for the 2048-wide layers

# Precision config (validated against the fp32 reference by CPU simulation
# and on hardware: single-f16 moving operands measure rel_err 1.45e-2 =
# 6/8192 label flips, well under the 2e-2 gate; hi/lo options kept for
# fallback)
L2_LO = False
L3_LO = False
SSQ_EXACT = False


def split_sync_waits(nc, max_waits=1):
    """Walrus here accepts at most `max_waits` sync waits per instruction.

    Tile emits instructions waiting on several semaphores at once.  For each
    such instruction, carry the excess waits on same-engine NoOps inserted
    immediately before it: the engine's sequencer executes them in order, so
    all waits still complete before the instruction runs (DMA instructions
    keep one wait in their descriptor; the NoOp just delays the ring).
    """
    uid = [0]
    for f in nc.m.functions:
        for bb in f.blocks:
            out = []
            changed = False
            for ins in bb.instructions:
                si = ins.sync_info
                if si is not None and len(si.on_wait) > max_waits:
                    waits = list(si.on_wait)
                    extra = waits[: len(waits) - max_waits]
                    for i in range(0, len(extra), max_waits):
                        uid[0] += 1
                        nop = mybir.InstNoOp(
                            name=f"waitsplit-{uid[0]}", engine=ins.engine,
                            ins=[], outs=[],
                        )
                        nop.sync_info = mybir.SyncInfo(
                            on_wait=extra[i : i + max_waits], on_update=[]
                        )
                        out.append(nop)
                    ins.sync_info = mybir.SyncInfo(
                        on_wait=waits[len(waits) - max_waits :],
                        on_update=list(si.on_update),
                    )
                    changed = True
                out.append(ins)
            if changed:
                bb.instructions = out


def build_nc(bc=BC, blk=BLK, h=H, l2_lo=L2_LO, l3_lo=L3_LO, ssq_exact=SSQ_EXACT):
    nblk = bc // blk
    ncol = blk // 128
    NOF = h // 128
    NK = h // 128
    nc = bass.Bass()

    xhi_d = nc.dram_tensor("xhi", [DP, bc], F16, kind="ExternalInput")
    xlo_d = nc.dram_tensor("xlo", [DP, bc], F16, kind="ExternalInput")
    s0_d = nc.dram_tensor("s0", [1, bc], F32, kind="ExternalInput")
    # w1g: [of, p, k, j] = W1.T[k*128+p, of*128+j] (per-of contiguous, padded)
    w1g_d = nc.dram_tensor("w1g", [NOF, 128, NKX, 128], F16, kind="ExternalInput")
    w2g_d = nc.dram_tensor("w2g", [NOF, 128, NK, 128], F16, kind="ExternalInput")
    w3g_d = nc.dram_tensor("w3g", [NOF, 128, NK, 128], F16, kind="ExternalInput")
    # w1ct[p, of*NL + l] = xmax * W1[of*128+p, l]  (fp32, exact)
    w1ct_d = nc.dram_tensor("w1ct", [128, NOF * NL], F32, kind="ExternalInput")
    b1_d = nc.dram_tensor("b1r", [NOF, 128], F32, kind="ExternalInput")
    b2_d = nc.dram_tensor("b2r", [NOF, 128], F32, kind="ExternalInput")
    b3_d = nc.dram_tensor("b3r", [NOF, 128], F32, kind="ExternalInput")
    out_d = nc.dram_tensor("out", [bc], I32, kind="ExternalOutput")

    from contextlib import ExitStack

    with tile.TileContext(nc) as tc:
        with ExitStack() as ctx:
            ec = ctx.enter_context
            wstr = ec(tc.tile_pool(name="wstr", bufs=3))
            w1o = ec(tc.tile_pool(name="w1o", bufs=2))
            xs = ec(tc.tile_pool(name="xs", bufs=1))
            const = ec(tc.tile_pool(name="const", bufs=1))
            ybs = ec(tc.tile_pool(name="ybs", bufs=1))
            hhp = ec(tc.tile_pool(name="hhp", bufs=6))
            hlp = ec(tc.tile_pool(name="hlp", bufs=2))
            f32t = ec(tc.tile_pool(name="f32t", bufs=2))
            hf = ec(tc.tile_pool(name="hf", bufs=2))
            h2s = ec(tc.tile_pool(name="h2s", bufs=2))
            sbc = ec(tc.tile_pool(name="sbc", bufs=3))
            s0p = ec(tc.tile_pool(name="s0p", bufs=2))
            zsp = ec(tc.tile_pool(name="zsp", bufs=4))
            srow = ec(tc.tile_pool(name="srow", bufs=2))
            srp = ec(tc.tile_pool(name="srp", bufs=4))
            glp = ec(tc.tile_pool(name="glp", bufs=4))
            gp = ec(tc.tile_pool(name="gp", bufs=1))
            outp = ec(tc.tile_pool(name="outp", bufs=2))
            zp = ec(tc.tile_pool(name="zp", bufs=4, space="PSUM"))
            ssqp = ec(tc.tile_pool(name="ssqp", bufs=4, space="PSUM"))
            # ---- constants --------------------------------------------------
            w1ct = const.tile([128, NOF, NL], F32, tag="w1ct")
            nc.sync.dma_start(
                w1ct[:, :, :], w1ct_d[:, :].rearrange("p (o l) -> p o l", l=NL)
            )
            bt = {}
            for li, bd in ((1, b1_d), (2, b2_d), (3, b3_d)):
                t = const.tile([128, NOF], F32, tag=f"b{li}")
                nc.gpsimd.dma_start(t[:, :], bd[:, :].rearrange("c p -> p c"))
                bt[li] = t
            ident16 = const.tile([16, 16], F32, tag="id16")
            make_identity(nc, ident16[:, :])
            ones_col = const.tile([128, 1], F16, tag="onec")
            nc.vector.memset(ones_col[:, :], 1.0)
            ones_row = const.tile([1, 128], F32, tag="oner")
            nc.vector.memset(ones_row[:, :], 1.0)
            desc_i = const.tile([128, 16], I32, tag="desci")
            nc.gpsimd.iota(desc_i[:, :], [[-1, 16]], base=16, channel_multiplier=0)
            desc_f = const.tile([128, 16], F32, tag="descf")
            nc.vector.tensor_copy(desc_f[:, :], desc_i[:, :])
            s0row = const.tile([1, bc], F32, tag="s0row")
            nc.sync.dma_start(s0row[:, :], s0_d[:, :])

            for blki in range(nblk):
                r0 = blki * blk

                # per-label state: sb (bcast scale), g_lab, h tiles
                st = {}

                def chain_to_sr(ssq):
                    """sr = 1/(sqrt(ssq)+eps) as a [1, blk] row (no broadcast)."""
                    u = srow.tile([1, blk], F32, tag="u")
                    nc.scalar.activation(u[:, :], ssq[:, :], AF.Sqrt)
                    ue = srow.tile([1, blk], F32, tag="ue")
                    nc.vector.tensor_scalar_add(ue[:, :], u[:, :], EPS)
                    sr = srp.tile([1, blk], F32, tag="sr")
                    nc.vector.reciprocal(sr[:, :], ue[:, :])
                    return sr

                def bcast_row(row_ap, pool=None):
                    """[1, blk] -> [128, blk] via ones-matmul; DVE drains PSUM.
                    Emit AFTER some independent PE work so the in-order PE does
                    not idle on the reciprocal chain."""
                    ps = zp.tile([128, blk], F32, tag="z", name="sbps")
                    nc.tensor.matmul(ps[:, :], ones_row[:, :], row_ap,
                                     start=True, stop=True)
                    sb = (pool or sbc).tile([128, blk], F32, tag="sb", name="sb")
                    nc.vector.tensor_copy(sb[:, :], ps[:, :])
                    return sb

                s0b = bcast_row(s0row[0:1, r0 : r0 + blk], pool=s0p)

                g_all = gp.tile([16, blk], F32, tag="gall")
                # rows 0..9 are overwritten per label; 10..15 stay -1e30
                nc.vector.memset(g_all[:, :], -1e30)

                def drain_and_square(lab, layer, of, tin, n_hi, n_lo, ssq):
                    """relu/cast/square pipeline shared by all layers.
                    tin: fp32 [128, blk] pre-bias pre-relu input (already scaled).
                    """
                    hv = hf.tile([128, blk], F32, tag="hv")
                    nc.scalar.activation(
                        hv[:, :], tin[:, :], AF.Relu,
                        bias=bt[layer][:, of : of + 1], scale=1.0,
                    )
                    if n_hi is not None:
                        nc.vector.tensor_copy(n_hi[:, of, :], hv[:, :])
                        if n_lo is not None:
                            nc.vector.tensor_tensor(
                                out=n_lo[:, of, :], in0=hv[:, :],
                                in1=n_hi[:, of, :], op=OP.subtract,
                            )
                    # ssq from the PRE-cast fp32 hv (precision-critical)
                    if ssq_exact:
                        h2 = f32t.tile([128, blk], F32, tag="h2")
                        nc.scalar.activation(h2[:, :], hv[:, :], AF.Square)
                        h2hi = h2s.tile([128, blk], F16, tag="h2hi")
                        nc.scalar.activation(h2hi[:, :], h2[:, :], AF.Copy)
                        h2lo = h2s.tile([128, blk], F16, tag="h2lo")
                        nc.vector.tensor_tensor(
                            out=h2lo[:, :], in0=h2[:, :], in1=h2hi[:, :],
                            op=OP.subtract,
                        )
                        nc.tensor.matmul(
                            ssq[:, :], ones_col[:, :], h2hi[:, :],
                            start=(of == 0), stop=False,
                        )
                        nc.tensor.matmul(
                            ssq[:, :], ones_col[:, :], h2lo[:, :],
                            start=False, stop=(of == NOF - 1),
                        )
                    else:
                        h2hi = h2s.tile([128, blk], F16, tag="h2hi")
                        nc.scalar.activation(h2hi[:, :], hv[:, :], AF.Square)
                        nc.tensor.matmul(
                            ssq[:, :], ones_col[:, :], h2hi[:, :],
                            start=(of == 0), stop=(of == NOF - 1),
                        )

                def l1_start(lab):
                    s = st[lab] = {}
                    s["n_hi"] = hhp.tile([128, NK, blk], F16, tag="hhi", name="n_hi")
                    s["n_lo"] = (
                        hlp.tile([128, NK, blk], F16, tag="hlo", name="n_lo")
                        if l2_lo
                        else None
                    )
                    s["ssq"] = ssqp.tile([1, blk], F32, tag="ssq", name="ssq")

                def l1_step(lab, of):
                    s = st[lab]
                    t0 = f32t.tile([128, blk], F32, tag="t0")
                    nc.vector.tensor_scalar(
                        out=t0[:, :], in0=ybt[:, of, :],
                        scalar1=w1ct[:, of, lab : lab + 1],
                        scalar2=None, op0=OP.add,
                    )
                    t = f32t.tile([128, blk], F32, tag="t")
                    nc.vector.tensor_tensor(
                        out=t[:, :], in0=t0[:, :], in1=s0b[:, :], op=OP.mult
                    )
                    drain_and_square(lab, 1, of, t, s["n_hi"], s["n_lo"], s["ssq"])

                def l1_finish(lab):
                    s = st[lab]
                    g_lab = glp.tile([1, blk], F32, tag="glab")
                    nc.vector.tensor_copy(g_lab[:, :], s["ssq"][:, :])
                    s["g"] = g_lab
                    s["sr"] = chain_to_sr(s["ssq"])

                # ---- ybase = W1 @ x_zeroed for this block, SBUF-resident ----
                # (layer 1 of label 0 rides along, one of-chunk at a time)
                ybt = ybs.tile([128, NOF, blk], F16, tag="ybt")
                xh, xl = [], []
                for ki in range(NKX):
                    th = xs.tile([128, blk], F16, tag=f"xh{ki}")
                    tl = xs.tile([128, blk], F16, tag=f"xl{ki}")
                    ko = ki * 128
                    nc.sync.dma_start(th[:, :], xhi_d[ko : ko + 128, r0 : r0 + blk])
                    nc.sync.dma_start(tl[:, :], xlo_d[ko : ko + 128, r0 : r0 + blk])
                    xh.append(th)
                    xl.append(tl)
                l1_start(0)
                l1_start(1)
                for of in range(NOF):
                    w1of = w1o.tile([128, NKX, 128], F16, tag="w1of")
                    nc.sync.dma_start(w1of[:, :, :], w1g_d[of, :, :, :])
                    ps = zp.tile([128, blk], F32, tag="z")
                    for ki in range(NKX):
                        nc.tensor.matmul(
                            ps[:, :], w1of[:, ki, :], xh[ki][:, :],
                            start=(ki == 0), stop=False,
                        )
                        nc.tensor.matmul(
                            ps[:, :], w1of[:, ki, :], xl[ki][:, :],
                            start=False, stop=(ki == NKX - 1),
                        )
                    nc.scalar.activation(ybt[:, of, :], ps[:, :], AF.Copy)
                    l1_step(0, of)
                    l1_step(1, of)
                l1_finish(0)
                l1_finish(1)

                # ---- labels in pairs: both share each stationary weight load,
                # so the PE streams back-to-back and LDWEIGHTS fully hides ----
                for pr in range(NL // 2):
                    A, B = 2 * pr, 2 * pr + 1
                    carry = pr + 1 < NL // 2
                    if carry:
                        l1_start(A + 2)
                        l1_start(B + 2)
                    hh = {A: st[A]["n_hi"], B: st[B]["n_hi"]}
                    hl = {A: st[A]["n_lo"], B: st[B]["n_lo"]}
                    for layer in (2, 3):
                        wg_d = w2g_d if layer == 2 else w3g_d
                        in_lo = (layer == 2 and l2_lo) or (layer == 3 and l3_lo)
                        out_lo = layer == 2 and l3_lo
                        nh = {A: None, B: None}
                        nl_ = {A: None, B: None}
                        if layer == 2:
                            for lab in (A, B):
                                t_ = hhp.tile(
                                    [128, NK, blk], F16, tag="hhi", name="n_hi"
                                )
                                nh[lab] = t_
                                if out_lo:
                                    t2_ = hlp.tile(
                                        [128, NK, blk], F16, tag="hlo", name="n_lo"
                                    )
                                    nl_[lab] = t2_
                        ssq = {}
                        for lab in (A, B):
                            ssq[lab] = ssqp.tile([1, blk], F32, tag="ssq", name="ssq")
                        sb = {A: None, B: None}
                        pend = []

                        def drain_one():
                            dof, dps = pend.pop(0)
                            for lab in (A, B):
                                t = f32t.tile([128, blk], F32, tag="t")
                                nc.vector.tensor_tensor(
                                    out=t[:, :], in0=dps[lab][:, :],
                                    in1=sb[lab][:, :], op=OP.mult,
                                )
                                drain_and_square(
                                    lab, layer, dof, t, nh[lab], nl_[lab], ssq[lab]
                                )

                        # carried layer-1 steps per of-iteration (the heavy
                        # of1/of2 iterations take fewer)
                        carry_plan = [3, 2, 1] + [2] * (NOF - 3)
                        carry_idx = 0
                        for of in range(NOF):
                            wof = wstr.tile([128, NK, 128], F16, tag="wof")
                            nc.sync.dma_start(wof[:, :, :], wg_d[of, :, :, :])
                            ps = {}
                            for lab in (A, B):
                                ps[lab] = zp.tile([128, blk], F32, tag="z", name="z")
                            for k in range(NK):
                                for lab in (A, B):
                                    nc.tensor.matmul(
                                        ps[lab][:, :], wof[:, k, :], hh[lab][:, k, :],
                                        start=(k == 0),
                                        stop=(k == NK - 1 and not in_lo),
                                    )
                                    if in_lo:
                                        nc.tensor.matmul(
                                            ps[lab][:, :], wof[:, k, :],
                                            hl[lab][:, k, :],
                                            start=False, stop=(k == NK - 1),
                                        )
                            if of < 2:
                                # copy PSUM->SBUF now: frees the banks without
                                # waiting for sb (whose chain is still running)
                                zs = {}
                                for lab in (A, B):
                                    zs[lab] = zsp.tile(
                                        [128, blk], F32, tag="zs", name="zs"
                                    )
                                    nc.vector.tensor_copy(
                                        zs[lab][:, :], ps[lab][:, :]
                                    )
                                pend.append((of, zs))
                            else:
                                pend.append((of, ps))
                            if of == 1:
                                # broadcast AFTER two paired of-groups (~14us):
                                # the in-order PE covers the reciprocal chains
                                # of both labels with real work
                                sb[A] = bcast_row(st[A]["sr"][0:1, :])
                                sb[B] = bcast_row(st[B]["sr"][0:1, :])
                            if of == 2:
                                drain_one()
                                drain_one()
                            elif of >= 3:
                                drain_one()
                            # layer-1 of the NEXT pair rides along in layer 2
                            # (so its ssq PSUM group closes before L3 starts)
                            if carry and layer == 2:
                                for _ in range(carry_plan[of]):
                                    if carry_idx < 2 * NOF:
                                        l1_step(
                                            A + 2 + (carry_idx % 2), carry_idx // 2
                                        )
                                        carry_idx += 1
                        while pend:
                            drain_one()
                        for lab in (A, B):
                            nc.vector.tensor_tensor(
                                out=st[lab]["g"][:, :], in0=st[lab]["g"][:, :],
                                in1=ssq[lab][:, :], op=OP.add,
                            )
                        if layer == 2:
                            for lab in (A, B):
                                st[lab]["sr"] = chain_to_sr(ssq[lab])
                            if carry:
                                l1_finish(A + 2)
                                l1_finish(B + 2)
                            hh, hl = nh, nl_
                    for lab in (A, B):
                        nc.gpsimd.dma_start(
                            g_all[lab : lab + 1, :], st[lab]["g"][:, :]
                        )
                        del st[lab]

                # ---- argmax over labels -------------------------------------
                for c in range(ncol):
                    cs = slice(c * 128, (c + 1) * 128)
                    gt = zp.tile([128, blk], F32, tag="z", name="gt")
                    nc.tensor.transpose(gt[:, 0:16], g_all[:, cs], ident16[:, :])
                    mx = outp.tile([128, 1], F32, tag="mx")
                    nc.vector.reduce_max(
                        out=mx[:, :], in_=gt[:, 0:16], axis=mybir.AxisListType.X
                    )
                    eq = outp.tile([128, 16], F32, tag="eq")
                    nc.vector.tensor_scalar(
                        out=eq[:, :], in0=gt[:, 0:16], scalar1=mx[:, :],
                        scalar2=None, op0=OP.is_equal,
                    )
                    wv = outp.tile([128, 16], F32, tag="wv")
                    nc.vector.tensor_tensor(
                        out=wv[:, :], in0=eq[:, :], in1=desc_f[:, :], op=OP.mult
                    )
                    vv = outp.tile([128, 1], F32, tag="vv")
                    nc.vector.reduce_max(
                        out=vv[:, :], in_=wv[:, :], axis=mybir.AxisListType.X
                    )
                    idx = outp.tile([128, 1], I32, tag="idx")
                    nc.vector.tensor_scalar(
                        out=idx[:, :], in0=vv[:, :], scalar1=-1.0, scalar2=16.0,
                        op0=OP.mult, op1=OP.add,
                    )
                    nc.sync.dma_start(
                        out_d[r0 + c * 128 : r0 + (c + 1) * 128], idx[:, :]
                    )
    split_sync_waits(nc)
    return nc


def prep_inputs(x, W1, b1, W2, b2, W3, b3, ncores=NCORES, h=H):
    """Host-side marshaling: overlay zeroing, norms of the fixed input rows,
    transposes, fp16 hi/lo split of x, weight regrouping, per-core sharding."""
    x = np.asarray(x, dtype=np.float32)
    bc = x.shape[0] // ncores
    xmax = np.float32(x.max())
    x_ = x.copy()
    x_[:, :NL] = 0.0
    ssq0 = (x_ * x_).sum(axis=1, dtype=np.float32) + xmax * xmax
    s0 = (np.float32(1.0) / (np.sqrt(ssq0) + np.float32(EPS))).astype(np.float32)

    xp = np.zeros((x.shape[0], DP), np.float32)
    xp[:, :D_IN] = x_
    xT = np.ascontiguousarray(xp.T)                      # [896, B] fp32
    xhi = xT.astype(np.float16)
    xlo = (xT - xhi.astype(np.float32)).astype(np.float16)

    NOF = NK = h // 128

    def regroup(Wt, nk):
        # Wt: [in, out] fp16 -> [of, p, k, j]
        return np.ascontiguousarray(
            Wt.reshape(nk, 128, NOF, 128).transpose(2, 1, 0, 3)
        )

    w1p = np.zeros((DP, h), np.float16)
    w1p[:D_IN] = W1.T.astype(np.float16)
    w1g = regroup(w1p, NKX)
    w2g = regroup(W2.T.astype(np.float16), NK)
    w3g = regroup(W3.T.astype(np.float16), NK)
    # w1ct[p, of, l] = xmax * W1[of*128+p, l], fp32 exact
    w1ct = np.ascontiguousarray(
        (xmax * W1[:, :NL].astype(np.float32)).reshape(NOF, 128, NL).transpose(1, 0, 2)
    ).reshape(128, NOF * NL)
    b1r = np.ascontiguousarray(b1.reshape(NOF, 128)).astype(np.float32)
    b2r = np.ascontiguousarray(b2.reshape(NOF, 128)).astype(np.float32)
    b3r = np.ascontiguousarray(b3.reshape(NOF, 128)).astype(np.float32)

    in_maps = []
    for c in range(ncores):
        rs = slice(c * bc, (c + 1) * bc)
        in_maps.append(
            {
                "xhi": np.ascontiguousarray(xhi[:, rs]),
                "xlo": np.ascontiguousarray(xlo[:, rs]),
                "s0": np.ascontiguousarray(s0[rs]).reshape(1, bc),
                "w1g": w1g, "w2g": w2g, "w3g": w3g, "w1ct": w1ct,
                "b1r": b1r, "b2r": b2r, "b3r": b3r,
            }
        )
    return in_maps, bc


_NC_CACHE = {}


def kernel(x, W1, b1, W2, b2, W3, b3, trace=False):
    in_maps, bc = prep_inputs(x, W1, b1, W2, b2, W3, b3)
    if "nc" not in _NC_CACHE:
        _NC_CACHE["nc"] = build_nc(bc=bc)
    res = run_bass_kernel_spmd(
        _NC_CACHE["nc"], in_maps, core_ids=list(range(NCORES)), trace=trace
    )
    out = np.concatenate([res.results[c]["out"] for c in range(NCORES)])
    if trace:
        kernel.last_results = res
    return out
